# revision 33
# baseline (speedup 1.0000x reference)
"""Trainium2 Bass kernel for AdaptiveMessagePassingLayer.

Math: out = X @ w_eff, where w_eff = sum_r scales[r] * relation_weights[r].
X: [524288, 128] f32, relation_weights: [16, 128, 128], relation_scales: [16, 1].

Sharding: data-parallel over the node dim N across 8 cores (65536 rows each).
Each shard is passed to its core transposed ([128, 65536], feature-major) so the
device streams K-major tiles straight into the TensorE with zero on-chip
transposes: out_shard.T = w_eff.T @ X_shard.T via matmul(lhsT=w_eff, rhs=xT).
The host transposes each core's [128, 65536] result back during unshard.
"""

import sys

if "/opt/trn_rl_repo" not in sys.path:
    sys.path.insert(0, "/opt/trn_rl_repo")

import numpy as np


def _ensure_axon_hooks():
    """The agent image lacks antenv.axon_hooks; bass_utils imports it when
    tracing is requested (e.g. BASS_TRACE=1). Register it with the NTFF
    profile hook so tracing works instead of crashing; degrade to a None
    hook if the boot helpers are unavailable."""
    try:
        import types

        import antenv

        if hasattr(antenv, "axon_hooks"):
            return
        mod = types.ModuleType("antenv.axon_hooks")
        _h = [None]
        mod.set_axon_ntff_profile_hook = lambda h: _h.__setitem__(0, h)
        mod.get_axon_ntff_profile_hook = lambda: _h[0]
        sys.modules["antenv.axon_hooks"] = mod
        antenv.axon_hooks = mod
        try:
            from trn_agent_boot.trn_boot import _ntff_profile_via_ctypes

            mod.set_axon_ntff_profile_hook(
                _ntff_profile_via_ctypes("/opt/axon/libaxon_pjrt.so"))
        except Exception:
            pass
    except Exception:
        pass


_ensure_axon_hooks()

import concourse.bass as bass
import concourse.tile as tile
from concourse import bacc, mybir
from concourse.bass_utils import run_bass_kernel_spmd

N_CORES = 8
N_NODES = 524288
D = 128
R = 16
M = N_NODES // N_CORES  # rows per core

BLK = 4096  # X rows (xT columns) per DMA block
MMT = 512   # moving-operand tile per fp32 matmul (hardware max)

_compiled = None


def build():
    f32 = mybir.dt.float32
    nc = bacc.Bacc("TRN2", target_bir_lowering=False, debug=False,
                   num_devices=N_CORES)
    xt = nc.dram_tensor("xt", [D, M], f32, kind="ExternalInput").ap()
    # rwt is relation_weights pre-rearranged on host to [i, r, o] so the
    # on-device load is one contiguous line-rate DMA (8KB per partition).
    rwt = nc.dram_tensor("rwt", [D, R, D], f32, kind="ExternalInput").ap()
    # scb is relation_scales replicated on host to [128, 16] (layout only)
    # so no cross-partition broadcast is needed on device.
    scb = nc.dram_tensor("scb", [D, R], f32, kind="ExternalInput").ap()
    out_t = nc.dram_tensor("out_t", [D, M], f32, kind="ExternalOutput").ap()

    mult = mybir.AluOpType.mult
    add = mybir.AluOpType.add

    with tile.TileContext(nc) as tc:
        with (
            tc.tile_pool(name="const", bufs=1) as const_pool,
            tc.tile_pool(name="inp", bufs=5) as inp,
            tc.tile_pool(name="outp", bufs=3) as outp,
            tc.tile_pool(name="ps", bufs=2, space="PSUM") as ps,
        ):
            # ---- w_eff = sum_r rs[r] * rw[r] ------------------------------
            # rw as [i, r, o]: partition i holds W[r, i, :] for every r.
            # First on the sync HWDGE ring: these land during the NEFF-start
            # window while the DMA engines are otherwise idle, so w_eff is
            # ready before the first input block finishes.
            sc_b = const_pool.tile([D, R], f32)
            nc.sync.dma_start(out=sc_b[:], in_=scb[:])
            wtile = const_pool.tile([D, R, D], f32)
            nc.sync.dma_start(out=wtile[:], in_=rwt[:])

            wscaled = const_pool.tile([D, R, D], f32)
            sc3d = sc_b[:].rearrange("i (r o) -> i r o", o=1)
            w3d, sc3d = bass.broadcast_tensor_aps(wtile[:], sc3d)
            nc.vector.tensor_tensor(out=wscaled[:], in0=w3d, in1=sc3d, op=mult)
            # Tree-reduce over r with contiguous wide adds (fast DVE mode).
            half = R
            while half > 1:
                half //= 2
                nc.vector.tensor_tensor(
                    out=wscaled[:, :half, :], in0=wscaled[:, :half, :],
                    in1=wscaled[:, half:2 * half, :], op=add)
            weff = wscaled[:, 0, :]

            # ---- main stream: out_t[:, c] = w_eff.T @ xt[:, c] ------------
            # Full blocks, then a tapered final block (short sub-blocks) so
            # the end-of-kernel in->matmul->copy->out drain tail is short.
            def do_span(col0, width, in_engine=None):
                xin = inp.tile([D, BLK], f32, tag="xin")
                xout = outp.tile([D, BLK], f32, tag="xout")
                for h0 in range(0, width, 2048):
                    hw = min(2048, width - h0)
                    (in_engine or nc.sync).dma_start(
                        out=xin[:, h0:h0 + hw],
                        in_=xt[:, col0 + h0:col0 + h0 + hw])
                # Fill a 4-bank PSUM tile with 4 matmuls, then drain it with
                # one wide DVE copy: per-block copy cost 2x2.29us instead of
                # 8x0.69us, keeping DVE ahead of the DMA block period.
                for g0 in range(0, width, 4 * MMT):
                    gw = min(4 * MMT, width - g0)
                    pt = ps.tile([D, 4 * MMT], f32, tag="pt")
                    for k0 in range(0, gw, MMT):
                        nc.tensor.matmul(
                            out=pt[:, k0:k0 + MMT], lhsT=weff[:],
                            rhs=xin[:, g0 + k0:g0 + k0 + MMT],
                            start=True, stop=True)
                    nc.vector.tensor_copy(out=xout[:, g0:g0 + gw],
                                          in_=pt[:, :gw])
                nc.scalar.dma_start(out=out_t[:, col0:col0 + width],
                                    in_=xout[:, :width])

            # Full blocks, then tapered tail; spans must cover exactly M cols.
            TAPER = [1024, 1024, 1024, 1024]
            spans = []
            remaining = M - sum(TAPER)
            while remaining >= BLK:
                spans.append(BLK)
                remaining -= BLK
            if remaining:
                spans.append(remaining)
            spans += TAPER
            assert sum(spans) == M and all(w % MMT == 0 for w in spans)
            col = 0
            for width in spans:
                do_span(col, width)
                col += width

    nc.compile()
    return nc


def kernel(inputs: np.ndarray, relation_weights: np.ndarray,
           relation_scales: np.ndarray) -> np.ndarray:
    global _compiled
    if _compiled is None:
        _compiled = build()
    nc = _compiled

    inputs = np.ascontiguousarray(inputs, dtype=np.float32)
    rwt = np.ascontiguousarray(
        np.asarray(relation_weights, dtype=np.float32).transpose(1, 0, 2))
    scb = np.ascontiguousarray(np.broadcast_to(
        np.asarray(relation_scales, dtype=np.float32).reshape(1, R), (D, R)))

    in_maps = []
    for i in range(N_CORES):
        shard_t = np.ascontiguousarray(inputs[i * M:(i + 1) * M].T)
        in_maps.append({"xt": shard_t, "rwt": rwt, "scb": scb})

    res = run_bass_kernel_spmd(nc, in_maps, core_ids=list(range(N_CORES)))

    out = np.empty((N_NODES, D), dtype=np.float32)
    for i in range(N_CORES):
        out[i * M:(i + 1) * M] = res.results[i]["out_t"].T
    return out
